# revision 19
# baseline (speedup 1.0000x reference)
"""Trainium2 Bass kernel v3: embedding gather + 2-layer MLP, data-parallel x8.

Same gather structure as the 215us baseline (128 serial INDIRECT1D calls/core,
one 128-row gather each — the HW consumes one index per partition and calls
serialize on the GpSimd engine at ~1.1us, so the call count is fixed), but:

  - fp16 table: gather moves 256B/row instead of 512B.
  - All 128 gathers land in one full-size SBUF buffer (no pool recycling), so
    the gather stream free-runs with no buffer-wait gaps between calls.
  - fp16 MLP datapath (PE transposes fp16 in, fp32 PSUM accumulate).
  - Output accumulated in SBUF as fp16 [86, 16384] and written in 8 slabs
    alternating sync/scalar HWDGE rings and gpsimd SWDGE queues, instead of
    32 f32 writes all landing on the 2 SDMA engines of the sync ring.

Host: flat index preprocessing and the same slot map as the baseline;
output upcast fp16->f32 + transpose per core.
"""

import numpy as np
from contextlib import ExitStack

import concourse.bass as bass
import concourse.bacc as bacc
import concourse.tile as tile
from concourse import mybir
from concourse.bass_utils import run_bass_kernel_spmd

F32 = mybir.dt.float32
F16 = mybir.dt.float16
I32 = mybir.dt.int32

N_CORES = 8
B = 131072
BC = B // N_CORES          # 16384 batch rows per core
FEAT = 128
NHID = 256
NOUT = 86
NROWS = 500000 * 4
P = 128
NJ = BC // P               # 128 j-columns of gathered rows per core
CHUNK_J = 4                # j-columns per MLP chunk (512 batch)
NCHUNK = NJ // CHUNK_J     # 32 chunks
SLAB = 2                   # chunks per output slab write (1024 cols)


def _build_program():
    nc = bacc.Bacc("TRN2", num_devices=N_CORES, num_swdge_queues=4)

    table = nc.dram_tensor("table", [NROWS, FEAT], F16, kind="ExternalInput").ap()
    idxs = nc.dram_tensor("idxs", [P, NJ], I32, kind="ExternalInput").ap()
    w1t = nc.dram_tensor("w1t", [FEAT, NHID], F16, kind="ExternalInput").ap()
    w2t = nc.dram_tensor("w2t", [P, NHID // P, NOUT], F16, kind="ExternalInput").ap()
    b1v = nc.dram_tensor("b1v", [P, NHID // P], F32, kind="ExternalInput").ap()
    b2v = nc.dram_tensor("b2v", [NOUT, 1], F32, kind="ExternalInput").ap()
    idtv = nc.dram_tensor("idtv", [P, P], F16, kind="ExternalInput").ap()
    outT = nc.dram_tensor("outT", [NOUT, BC], F16, kind="ExternalOutput").ap()

    with tile.TileContext(nc) as tc, ExitStack() as ctx:
        const = ctx.enter_context(tc.tile_pool(name="const", bufs=1))
        xpool = ctx.enter_context(tc.tile_pool(name="xt", bufs=3))
        hpool = ctx.enter_context(tc.tile_pool(name="ht", bufs=3))
        psum = ctx.enter_context(tc.tile_pool(name="psum", bufs=2, space="PSUM"))

        idx_t = const.tile([P, NJ], I32)
        for k in range(4):
            ks = slice(k * (NJ // 4), (k + 1) * (NJ // 4))
            nc.sync.dma_start(idx_t[:, ks], idxs[:, ks])

        idt = const.tile([P, P], F16)
        nc.scalar.dma_start(idt[:], idtv[:])

        w1t_t = const.tile([FEAT, NHID], F16)
        nc.scalar.dma_start(w1t_t[:], w1t[:])
        w2t_t = const.tile([P, NHID // P, NOUT], F16)
        nc.scalar.dma_start(w2t_t[:], w2t[:])
        b1_t = const.tile([P, NHID // P], F32)
        nc.scalar.dma_start(b1_t[:], b1v[:])
        b2_t = const.tile([NOUT, 1], F32)
        nc.scalar.dma_start(b2_t[:], b2v[:])

        # one resident gather buffer: 128 j-cols x 128 feat fp16 = 32KB/part
        xbuf = const.tile([P, NJ, FEAT], F16)
        # resident output accumulator [86, 16384] fp16 = 32KB/part
        obuf = const.tile([NOUT, BC], F16)

        for j in range(NJ):
            g = nc.gpsimd.indirect_dma_start(
                out=xbuf[:, j, :],
                out_offset=None,
                in_=table[:],
                in_offset=bass.IndirectOffsetOnAxis(ap=idx_t[:, j:j + 1], axis=0),
            )
            g.ins.queue = f"qPoolDynamic{j % 4 or ''}"
            # 256B descriptors: packetized generation (concat consecutive
            # descs) — the documented win regime is desc size <= 256B
            g.ins.single_packet = True

        # full-size chunks, then a tapered tail so the post-gather drain
        # chain (transpose -> copy -> MMs -> acts -> write) is short
        chunks = [(c * CHUNK_J, CHUNK_J) for c in range(31)] + \
                 [(124, 2), (126, 1), (127, 1)]
        for c, (jb, nj) in enumerate(chunks):
            w = nj * P
            bcol = jb * P
            xtr = psum.tile([P, CHUNK_J * P], F16, tag="xtr")
            for i in range(nj):
                nc.tensor.transpose(
                    out=xtr[:, i * P:(i + 1) * P],
                    in_=xbuf[:, jb + i, :],
                    identity=idt[:],
                )
            xt = xpool.tile([P, CHUNK_J * P], F16)
            nc.vector.tensor_copy(out=xt[:, :w], in_=xtr[:, :w])

            hp = psum.tile([P, NHID // P, CHUNK_J * P], F32, tag="h")
            for k in range(NHID // P):
                nc.tensor.matmul(
                    out=hp[:, k, :w],
                    lhsT=w1t_t[:, k * P:(k + 1) * P],
                    rhs=xt[:, :w],
                    start=True, stop=True,
                )
            ht = hpool.tile([P, NHID // P, CHUNK_J * P], F16)
            nc.scalar.activation(
                out=ht[:, 0, :w], in_=hp[:, 0, :w],
                func=mybir.ActivationFunctionType.Relu,
                bias=b1_t[:, 0:1],
            )
            nc.vector.tensor_scalar(
                out=ht[:, 1, :w], in0=hp[:, 1, :w],
                scalar1=b1_t[:, 1:2], scalar2=0.0,
                op0=mybir.AluOpType.add, op1=mybir.AluOpType.max,
            )

            op_ = psum.tile([NOUT, CHUNK_J * P], F32, tag="ot")
            for k in range(NHID // P):
                nc.tensor.matmul(
                    out=op_[:, :w],
                    lhsT=w2t_t[:, k, :],
                    rhs=ht[:, k, :w],
                    start=(k == 0), stop=(k == NHID // P - 1),
                )
            nc.scalar.activation(
                out=obuf[:, bcol:bcol + w], in_=op_[:, :w],
                func=mybir.ActivationFunctionType.Relu,
                bias=b2_t[:],
            )

            if c >= 30:
                # drain tail chunks individually on parallel rings
                eng = nc.sync if c % 2 == 0 else nc.scalar
                eng.dma_start(outT[:, bcol:bcol + w], obuf[:, bcol:bcol + w])
            elif c % SLAB == SLAB - 1:
                lo = (c - SLAB + 1) * CHUNK_J * P
                hi = (c + 1) * CHUNK_J * P
                s = c // SLAB
                eng = nc.sync if s % 2 == 0 else nc.scalar
                eng.dma_start(outT[:, lo:hi], obuf[:, lo:hi])

    nc.compile()
    return nc


TRACE = False
RUN_KWARGS = None
LAST = None

_SLOT_TO_BATCH = None


def _slot_map():
    global _SLOT_TO_BATCH
    if _SLOT_TO_BATCH is None:
        pp, jj = np.meshgrid(np.arange(P), np.arange(NJ), indexing="ij")
        cc = jj // CHUNK_J
        ii = jj % CHUNK_J
        _SLOT_TO_BATCH = cc * (CHUNK_J * P) + ii * P + pp  # [P, NJ]
    return _SLOT_TO_BATCH


def kernel(entity_embedding, w1, b1, w2, b2, idx0, idx1):
    table = np.asarray(entity_embedding, dtype=np.float32) \
        .reshape(NROWS, FEAT).astype(np.float16)
    flat_idx = (np.asarray(idx0, dtype=np.int64) * 4
                + np.asarray(idx1, dtype=np.int64)).astype(np.int32)
    w1tf = np.ascontiguousarray(np.asarray(w1, dtype=np.float32).T) \
        .astype(np.float16)
    w2tf = np.ascontiguousarray(
        np.asarray(w2, dtype=np.float32).T.reshape(NHID // P, P, NOUT)
        .transpose(1, 0, 2)).astype(np.float16)
    b1v = np.ascontiguousarray(
        np.asarray(b1, dtype=np.float32).reshape(NHID // P, P).T)
    b2v = np.ascontiguousarray(np.asarray(b2, dtype=np.float32).reshape(NOUT, 1))

    slot = _slot_map()
    in_maps = []
    for core in range(N_CORES):
        local = flat_idx[core * BC:(core + 1) * BC]
        idxs = np.ascontiguousarray(local[slot])  # [P, NJ] int32
        in_maps.append({
            "table": table,
            "idxs": idxs,
            "w1t": w1tf,
            "w2t": w2tf,
            "b1v": b1v,
            "b2v": b2v,
            "idtv": np.eye(P, dtype=np.float16),
        })

    nc = _build_program()
    global LAST
    res = run_bass_kernel_spmd(
        nc, in_maps, core_ids=list(range(N_CORES)), trace=TRACE,
        **(RUN_KWARGS or {}),
    )
    LAST = res
    out = np.empty((B, NOUT), dtype=np.float32)
    for core in range(N_CORES):
        out[core * BC:(core + 1) * BC] = \
            np.asarray(res.results[core]["outT"], dtype=np.float32).T
    return out
